# revision 50
# baseline (speedup 1.0000x reference)
"""GaussianImage (Cholesky) renderer on 8 trn2 NeuronCores.

Strategy: tile-parallel over the pixel grid with multi-tile slot packing.
The 256x256 image is cut into 32x32-pixel tiles (64/frame, 128 total for
T=2).  The host bins gaussians to tiles (bbox intersect via a
conservative support radius; outside it exp(-sigma) underflows to 0 in
fp32) and then bin-packs several tiles into one 128-slot "group"
(occupancies sum to <= 128; mean tile occupancy is ~40).  All tiles share
the same local 32x32 quadratic basis, so one K=12 fp16 matmul + one Exp
evaluates every gaussian of every tile in the group against all 1024
local pixels:

  sigma = lhsT(12,128)^T @ basis(12,1024)      [TensorE fp16, fp32 PSUM]
  alpha = Exp(-sigma)                          [ScalarE -> fp16]
  img   = w(128,32)^T @ alpha(128,1024)        [TensorE fp16, fp32 PSUM]

The img weights are block-structured: column 3j+c holds channel-c colors
of tile j's gaussians at their slots and zeros elsewhere, so one matmul
renders every tile of the group (row 3j+c = tile j, channel c).  Per-slot
sigma coefficients (quadratic in local pixel coords) are precomputed on
the host and split hi/lo into two fp16 values (lo pre-scaled by 2^11 to
stay in fp16 normal range; matching basis rows scaled by 2^-11), giving
~fp32-accurate sigma at the PE's full fp16 rate (fp32 matmul is 2-4x
slower on the PE; fp16 with exact-integer basis values loses nothing).
Up to 4 groups' images land in per-bank PSUM tiles at partition offsets
32i (PE column-group tiling), are clamped+converted to fp16 by one
VectorE op per 512-col half, and DMA'd out.  Inputs ride four parallel
engine DMA queues; the Exp table is preloaded during the transfers.
Each pixel is owned by exactly one tile -> no cross-core reduction.
"""

import os
import numpy as np

T, N, H, W = 2, 512, 256, 256
TILE = 32
NT = H // TILE          # 8 tiles per axis
N_CORES = 8
SLOTS = 128
PIX = TILE * TILE       # 1024
MAXTILES = 10           # 3*MAXTILES <= 32 img-weight columns per group
SIGMA_CUT = 8.0         # exp(-8) ~ 3e-4 * w: ~1e-3 rel, vs the 2e-2 gate
LO_SCALE = 2048.0       # 2^11: keeps lo-half fp16 coefficients normal

_CACHE = {}


def _build_nc(E):
    import concourse.bass as bass
    import concourse.mybir as mybir
    from concourse.tile import TileContext
    import bass_rust

    f32 = mybir.dt.float32
    f16 = mybir.dt.float16
    Act = mybir.ActivationFunctionType
    Alu = mybir.AluOpType
    G = (E + 3) // 4  # img PSUM tiles, 4 groups each

    nc = bass.Bass("TRN2")
    # Inputs split across three engine DMA queues so the transfers run in
    # parallel right after engine init, and the first sig matmul's inputs
    # (basis half A + group-0 coeffs) ride the two queue HEADS (DMA issue
    # time scales with bytes, so the critical first transfers are small):
    #   sync   queue: [basis cols 0:512] then [groups 1.. coeffs]
    #   gpsimd queue: [group-0 coeffs] then [basis cols 512:1024]
    #   scalar queue: [img weights]
    basa = nc.dram_tensor("basa", [12, 512], f16, kind="ExternalInput")
    cb0 = nc.dram_tensor("cb0", [12, SLOTS], f16, kind="ExternalInput")
    cbs = nc.dram_tensor("cbs", [12, 512], f16, kind="ExternalInput")
    cbb = nc.dram_tensor("cbb", [12, max(E - 1, 1) * SLOTS], f16,
                         kind="ExternalInput")
    wtr = nc.dram_tensor("wtr", [SLOTS, E * 32], f16, kind="ExternalInput")
    out = nc.dram_tensor("out", [G, SLOTS, PIX], f16, kind="ExternalOutput")

    with TileContext(nc) as tc:
        with tc.tile_pool(name="const", bufs=1) as cpool, \
             tc.tile_pool(name="alpha", bufs=4) as apool, \
             tc.tile_pool(name="ps_sig", bufs=3 if G == 1 else 2,
                          space="PSUM") as pss, \
             tc.tile_pool(name="ps_img", bufs=2 if G > 1 else 1,
                          space="PSUM") as psi:

            S = nc.scalar

            # preload the Exp activation table first thing on the scalar
            # queue (the ~1.3us table load otherwise serializes with the
            # first real exp).  The warm tile is read uninitialized on
            # purpose: its output is never consumed.
            warm = cpool.tile([SLOTS, 1], f32, tag="warm")
            S.activation(warm, warm, Act.Exp)

            bta = cpool.tile([12, 512], f16, tag="basa")
            ct0 = cpool.tile([12, SLOTS], f16, tag="cb0")
            cts = cpool.tile([12, 512], f16, tag="cbs")
            ctb = cpool.tile([12, max(E - 1, 1) * SLOTS], f16, tag="cbb")
            wt = cpool.tile([SLOTS, E * 32], f16, tag="wtr")
            nc.scalar.dma_start(out=wt, in_=wtr[:])
            nc.sync.dma_start(out=bta, in_=basa[:])
            nc.gpsimd.dma_start(out=ct0, in_=cb0[:])
            nc.gpsimd.dma_start(out=cts, in_=cbs[:])
            nc.sync.dma_start(out=ctb, in_=cbb[:])

            def bt(c0, c1):
                return bta[:, c0:c1] if c1 <= 512 else cts[:, c0 - 512:c1 - 512]

            alphas = {}
            imgt = {}

            def emit_img(e):
                g, i = divmod(e, 4)
                al = alphas.pop(e)
                wre = wt[:, 32 * e:32 * e + 32]
                if i == 0:
                    # separate single-bank PSUM tiles per 512-col half so
                    # the drain of one half never serializes against img
                    # matmuls of the other (the bank tracker is per-tile)
                    imgt[g] = (psi.tile([SLOTS, 512], f32, tag="imgA",
                                        name=f"imgA{g}"),
                               psi.tile([SLOTS, 512], f32, tag="imgB",
                                        name=f"imgB{g}"))
                final = e == E - 1
                last = final or i == 3
                for h, (c0, c1) in enumerate(((0, 512), (512, 1024))):
                    t = imgt[g][h]
                    nc.tensor.matmul(t[32 * i:32 * i + 32, :], wre,
                                     al[:, c0:c1], start=True, stop=True,
                                     tile_position=(0, 32 * i))
                    if last:
                        # fused clamp + PSUM->SBUF fp16 copy per half (one
                        # DVE op covers all groups in the block: cost is
                        # cols, not rows), then DMA out only the used rows
                        rows = 32 * (i + 1)
                        st = apool.tile([SLOTS, 512], f16, tag=f"st{h}",
                                        name=f"st{g}_{h}")
                        nc.vector.tensor_scalar(out=st[0:rows, :],
                                                in0=t[0:rows, :],
                                                scalar1=0.0, scalar2=1.0,
                                                op0=Alu.max, op1=Alu.min)
                        nc.sync.dma_start(out=out[g][0:rows, c0:c1],
                                          in_=st[0:rows, :])

            for e in range(E):
                sig = pss.tile([SLOTS, PIX], f32, tag="sig", name=f"sig{e}")
                lh = ct0 if e == 0 else ctb[:, SLOTS * (e - 1):SLOTS * e]
                nc.tensor.matmul(sig[:, 0:512], lh, bt(0, 512),
                                 start=True, stop=True)
                nc.tensor.matmul(sig[:, 512:1024], lh, bt(512, 1024),
                                 start=True, stop=True)
                # software pipeline with a lag of 2 groups: by the time the
                # img matmuls of group e-2 issue, its exp finished long ago,
                # so the PE never stalls on the ScalarE
                if e >= 2:
                    emit_img(e - 2)
                al = apool.tile([SLOTS, PIX], f16, tag="alpha", name=f"al{e}")
                if e == 0 or e == E - 1:
                    # split the first exp (starts right after the first sig
                    # half-matmul, pulling the whole ScalarE chain earlier)
                    # and the final exp (pipelines the drain chain
                    # sig->exp->img->clamp->dma at 512-col grain)
                    S.activation(al[:, 0:512], sig[:, 0:512], Act.Exp,
                                 scale=-1.0)
                    S.activation(al[:, 512:1024], sig[:, 512:1024], Act.Exp,
                                 scale=-1.0)
                else:
                    S.activation(al, sig, Act.Exp, scale=-1.0)
                alphas[e] = al
            if E >= 2:
                emit_img(E - 2)
            emit_img(E - 1)

    bass_rust.generate_event_semaphores(nc)
    return nc


def _bin_entries(cx, cy, lam):
    """Host-side routing: which gaussians overlap which 32x32 tile."""
    r = np.sqrt(2.0 * SIGMA_CUT * np.maximum(lam, 0.0)) + 1.0

    entries = []  # (frame, ty, tx, index-list)
    for t in range(T):
        x0 = np.clip(((cx[t] - r[t]) // TILE).astype(int), 0, NT - 1)
        x1 = np.clip(((cx[t] + r[t]) // TILE).astype(int), 0, NT - 1)
        y0 = np.clip(((cy[t] - r[t]) // TILE).astype(int), 0, NT - 1)
        y1 = np.clip(((cy[t] + r[t]) // TILE).astype(int), 0, NT - 1)
        buckets = [[[] for _ in range(NT)] for _ in range(NT)]
        for n in range(N):
            for ty in range(y0[n], y1[n] + 1):
                for tx in range(x0[n], x1[n] + 1):
                    buckets[ty][tx].append(n)
        for ty in range(NT):
            for tx in range(NT):
                assert len(buckets[ty][tx]) <= SLOTS, "tile overflow: >128 gaussians"
                entries.append((t, ty, tx, buckets[ty][tx]))
    return entries


def _pack_bins(entries, bins_per_core):
    """Pack tile-entries into groups of <= 128 total slots, <= MAXTILES
    tiles, load-balanced over N_CORES * bins_per_core bins. Returns a list
    of bins, each a list of entry indices, or None if infeasible."""
    nbins = N_CORES * bins_per_core
    order = sorted(range(len(entries)),
                   key=lambda k: -len(entries[k][3]))
    loads = [0] * nbins
    counts = [0] * nbins
    bins = [[] for _ in range(nbins)]
    for k in order:
        occ = len(entries[k][3])
        best = -1
        for b in sorted(range(nbins), key=lambda b: loads[b]):
            if counts[b] < MAXTILES and loads[b] + occ <= SLOTS:
                best = b
                break
        if best < 0:
            return None
        bins[best].append(k)
        loads[best] += occ
        counts[best] += 1
    return bins


def _ensure_ntff_hook():
    """Provide antenv.axon_hooks (missing in this image) so trace=True works."""
    import sys, types, ctypes, contextlib
    if "antenv.axon_hooks" in sys.modules:
        return
    so_path = "/opt/axon/libaxon_pjrt.so"
    if not os.path.exists(so_path):
        return
    lib = ctypes.CDLL(so_path)
    if not hasattr(lib, "axon_start_nrt_profile"):
        return
    lib.axon_start_nrt_profile.argtypes = [ctypes.POINTER(ctypes.c_int64), ctypes.c_size_t]
    lib.axon_start_nrt_profile.restype = ctypes.c_int64
    lib.axon_stop_nrt_profile.argtypes = [ctypes.c_char_p]
    lib.axon_stop_nrt_profile.restype = ctypes.c_int64

    @contextlib.contextmanager
    def _hook(output_dir, device_ids):
        import jax
        jax.devices()
        if device_ids:
            ids = (ctypes.c_int64 * len(device_ids))(*device_ids)
            rc = lib.axon_start_nrt_profile(ids, len(device_ids))
        else:
            rc = lib.axon_start_nrt_profile(None, 0)
        if rc != 0:
            raise RuntimeError(f"axon_start_nrt_profile rc={rc}")
        try:
            yield
        finally:
            n = lib.axon_stop_nrt_profile(str(output_dir).encode())
            print(f"profile: {n} file(s) written to {output_dir}")

    mod = types.ModuleType("antenv.axon_hooks")
    mod.get_axon_ntff_profile_hook = lambda: _hook
    mod.set_axon_ntff_profile_hook = lambda h: None
    sys.modules["antenv.axon_hooks"] = mod


def _split16(c):
    """Split float64 array c into (hi, lo) fp16 with lo pre-scaled by 2^11."""
    hi = c.astype(np.float16)
    lo = ((c - hi.astype(np.float64)) * LO_SCALE).astype(np.float16)
    return hi, lo


def kernel(xyz, cholesky, opacity, features_dc):
    from concourse import bass_utils

    xyz = np.asarray(xyz, np.float32)
    cholesky = np.asarray(cholesky, np.float32)
    opacity = np.asarray(opacity, np.float32)
    features_dc = np.asarray(features_dc, np.float32)

    # ---- host precompute (float64): projection, conic, binning ----
    means = np.tanh(xyz.astype(np.float64))
    cx = 0.5 * W * (means[..., 0] + 1.0)                    # (T,N)
    cy = 0.5 * H * (means[..., 1] + 1.0)
    chol = cholesky.astype(np.float64) + np.array([0.5, 0.0, 0.5])
    l0, l1, l2 = chol[..., 0], chol[..., 1], chol[..., 2]
    sxx, sxy, syy = l0 * l0, l0 * l1, l1 * l1 + l2 * l2
    det = sxx * syy - sxy * sxy
    ca, cb, cc = syy / det, -sxy / det, sxx / det           # conic (T,N)
    tr = sxx + syy
    lam = tr / 2 + np.sqrt(np.maximum(tr * tr / 4 - det, 0.0))

    colors = 1.0 / (1.0 + np.exp(-features_dc.astype(np.float64)))   # (N,3)
    opac = 1.0 / (1.0 + np.exp(-opacity.astype(np.float64)[:, 0]))   # (N,)
    w3 = colors * opac[:, None]                                      # (N,3)

    entries = _bin_entries(cx, cy, lam)
    total = sum(len(e[3]) for e in entries)
    E = max(2, -(-total // (SLOTS * N_CORES)))   # bins per core, lower bound
    bins = None
    while bins is None:
        bins = _pack_bins(entries, E)
        if bins is None:
            E += 1

    # fp16 quadratic basis over local 32x32 pixels; rows 6-11 are the
    # lo-coefficient rows, scaled by 2^-11 (power of two: still exact)
    gx = np.arange(PIX, dtype=np.float64) % TILE
    gy = np.arange(PIX, dtype=np.float64) // TILE
    b6 = np.stack([gx * gx, gx * gy, gy * gy, gx, gy, np.ones(PIX)])
    basis = np.concatenate([b6, b6 / LO_SCALE]).astype(np.float16)

    in_maps = []
    for c in range(N_CORES):
        lm = np.zeros((12, E * SLOTS), np.float16)
        wm = np.zeros((SLOTS, E * 32), np.float16)
        for ei in range(E):
            off = 0
            for j, k in enumerate(bins[c * E + ei]):
                t, ty, tx, idxs = entries[k]
                ns = len(idxs)
                if not ns:
                    continue
                idxs = np.asarray(idxs)
                ex = cx[t, idxs] - tx * TILE
                ey = cy[t, idxs] - ty * TILE
                a_, b_, c_ = ca[t, idxs], cb[t, idxs], cc[t, idxs]
                coef = np.stack([
                    0.5 * a_,
                    b_,
                    0.5 * c_,
                    -(a_ * ex + b_ * ey),
                    -(b_ * ex + c_ * ey),
                    0.5 * (a_ * ex * ex + c_ * ey * ey) + b_ * ex * ey,
                ])                                           # (6, ns)
                hi, lo = _split16(coef)
                s = slice(SLOTS * ei + off, SLOTS * ei + off + ns)
                lm[0:6, s] = hi
                lm[6:12, s] = lo
                wm[off:off + ns, 32 * ei + 3 * j:32 * ei + 3 * j + 3] = \
                    w3[idxs].astype(np.float16)
                off += ns
        cbbm = np.ascontiguousarray(lm[:, SLOTS:]) if E > 1 else \
            np.zeros((12, SLOTS), np.float16)
        in_maps.append({"basa": np.ascontiguousarray(basis[:, 0:512]),
                        "cb0": np.ascontiguousarray(lm[:, 0:SLOTS]),
                        "cbs": np.ascontiguousarray(basis[:, 512:1024]),
                        "cbb": cbbm, "wtr": wm})

    if E not in _CACHE:
        _CACHE[E] = _build_nc(E)
    nc = _CACHE[E]

    trace = bool(int(os.environ.get("GS_TRACE", "0")))
    if trace:
        _ensure_ntff_hook()
    res = bass_utils.run_bass_kernel_spmd(
        nc, in_maps, core_ids=list(range(N_CORES)), trace=trace)
    kernel.last_result = res

    img = np.zeros((T, 3, H, W), np.float32)
    for c in range(N_CORES):
        o = res.results[c]["out"]     # (G, 128, PIX) fp16
        for ei in range(E):
            g, i = divmod(ei, 4)
            for j, k in enumerate(bins[c * E + ei]):
                t, ty, tx, _ = entries[k]
                blk = o[g, 32 * i + 3 * j:32 * i + 3 * j + 3]
                img[t, :, ty * TILE:(ty + 1) * TILE,
                    tx * TILE:(tx + 1) * TILE] = \
                    blk.reshape(3, TILE, TILE)
    return np.clip(img, 0.0, 1.0)


# revision 54
# speedup vs baseline: 1.0287x; 1.0287x over previous
"""GaussianImage (Cholesky) renderer on 8 trn2 NeuronCores.

Strategy: tile-parallel over the pixel grid with multi-tile slot packing.
The 256x256 image is cut into 32x32-pixel tiles (64/frame, 128 total for
T=2).  The host bins gaussians to tiles (bbox intersect via a
conservative support radius; outside it exp(-sigma) underflows to 0 in
fp32) and then bin-packs several tiles into one 128-slot "group"
(occupancies sum to <= 128; mean tile occupancy is ~40).  All tiles share
the same local 32x32 quadratic basis, so one K=12 fp16 matmul + one Exp
evaluates every gaussian of every tile in the group against all 1024
local pixels:

  sigma = lhsT(12,128)^T @ basis(12,1024)      [TensorE fp16, fp32 PSUM]
  alpha = Exp(-sigma)                          [ScalarE -> fp16]
  img   = w(128,32)^T @ alpha(128,1024)        [TensorE fp16, fp32 PSUM]

The img weights are block-structured: column 3j+c holds channel-c colors
of tile j's gaussians at their slots and zeros elsewhere, so one matmul
renders every tile of the group (row 3j+c = tile j, channel c).  Per-slot
sigma coefficients (quadratic in local pixel coords) are precomputed on
the host and split hi/lo into two fp16 values (lo pre-scaled by 2^11 to
stay in fp16 normal range; matching basis rows scaled by 2^-11), giving
~fp32-accurate sigma at the PE's full fp16 rate (fp32 matmul is 2-4x
slower on the PE; fp16 with exact-integer basis values loses nothing).
Up to 4 groups' images land in per-bank PSUM tiles at partition offsets
32i (PE column-group tiling), are clamped+converted to fp16 by one
VectorE op per 512-col half, and DMA'd out.  Inputs ride four parallel
engine DMA queues; the Exp table is preloaded during the transfers.
Each pixel is owned by exactly one tile -> no cross-core reduction.
"""

import os
import numpy as np

T, N, H, W = 2, 512, 256, 256
TILE = 32
NT = H // TILE          # 8 tiles per axis
N_CORES = 8
SLOTS = 128
PIX = TILE * TILE       # 1024
MAXTILES = 10           # 3*MAXTILES <= 32 img-weight columns per group
SIGMA_CUT = 8.0         # exp(-8) ~ 3e-4 * w: ~1e-3 rel, vs the 2e-2 gate
LO_SCALE = 2048.0       # 2^11: keeps lo-half fp16 coefficients normal

_CACHE = {}


def _build_nc(E):
    import concourse.bass as bass
    import concourse.mybir as mybir
    from concourse.tile import TileContext
    import bass_rust

    f32 = mybir.dt.float32
    f16 = mybir.dt.float16
    Act = mybir.ActivationFunctionType
    Alu = mybir.AluOpType
    G = (E + 3) // 4  # img PSUM tiles, 4 groups each

    nc = bass.Bass("TRN2")
    # Inputs split into four tensors so each rides a different engine's
    # DMA queue and the transfers run in parallel right after engine init:
    #   cba: [basis cols 0:512 | group-0 lhsT coeffs]   (sync queue)
    #   cbs: [basis cols 512:1024]                      (gpsimd queue)
    #   cbb: remaining groups' lhsT coeffs              (sync queue, 2nd)
    #   wtr: img weights                                (scalar queue)
    cba = nc.dram_tensor("cba", [12, 512 + SLOTS], f16, kind="ExternalInput")
    cbs = nc.dram_tensor("cbs", [12, 512], f16, kind="ExternalInput")
    cbb = nc.dram_tensor("cbb", [12, max(E - 1, 1) * SLOTS], f16,
                         kind="ExternalInput")
    wtr = nc.dram_tensor("wtr", [SLOTS, E * 32], f16, kind="ExternalInput")
    out = nc.dram_tensor("out", [G, SLOTS, PIX], f16, kind="ExternalOutput")

    with TileContext(nc) as tc:
        with tc.tile_pool(name="const", bufs=1) as cpool, \
             tc.tile_pool(name="alpha", bufs=4) as apool, \
             tc.tile_pool(name="ps_sig", bufs=3 if G == 1 else 2,
                          space="PSUM") as pss, \
             tc.tile_pool(name="ps_img", bufs=2 if G > 1 else 1,
                          space="PSUM") as psi:

            S = nc.scalar

            # preload the Exp activation table first thing on the scalar
            # queue (the ~1.3us table load otherwise serializes with the
            # first real exp).  The warm tile is read uninitialized on
            # purpose: its output is never consumed.
            warm = cpool.tile([SLOTS, 1], f32, tag="warm")
            S.activation(warm, warm, Act.Exp)

            cta = cpool.tile([12, 512 + SLOTS], f16, tag="cba")
            cts = cpool.tile([12, 512], f16, tag="cbs")
            ctb = cpool.tile([12, max(E - 1, 1) * SLOTS], f16, tag="cbb")
            wt = cpool.tile([SLOTS, E * 32], f16, tag="wtr")
            nc.scalar.dma_start(out=wt, in_=wtr[:])
            nc.sync.dma_start(out=cta, in_=cba[:])
            nc.gpsimd.dma_start(out=cts, in_=cbs[:])
            nc.sync.dma_start(out=ctb, in_=cbb[:])

            def bt(c0, c1):
                return cta[:, c0:c1] if c1 <= 512 else cts[:, c0 - 512:c1 - 512]

            alphas = {}
            imgt = {}

            def emit_img(e):
                g, i = divmod(e, 4)
                al = alphas.pop(e)
                wre = wt[:, 32 * e:32 * e + 32]
                if i == 0:
                    # separate single-bank PSUM tiles per 512-col half so
                    # the drain of one half never serializes against img
                    # matmuls of the other (the bank tracker is per-tile)
                    imgt[g] = (psi.tile([SLOTS, 512], f32, tag="imgA",
                                        name=f"imgA{g}"),
                               psi.tile([SLOTS, 512], f32, tag="imgB",
                                        name=f"imgB{g}"))
                final = e == E - 1
                last = final or i == 3
                for h, (c0, c1) in enumerate(((0, 512), (512, 1024))):
                    t = imgt[g][h]
                    nc.tensor.matmul(t[32 * i:32 * i + 32, :], wre,
                                     al[:, c0:c1], start=True, stop=True,
                                     tile_position=(0, 32 * i))
                    if last:
                        # fused clamp + PSUM->SBUF fp16 copy per half (one
                        # DVE op covers all groups in the block: cost is
                        # cols, not rows), then DMA out only the used rows
                        rows = 32 * (i + 1)
                        st = apool.tile([SLOTS, 512], f16, tag=f"st{h}",
                                        name=f"st{g}_{h}")
                        nc.vector.tensor_scalar(out=st[0:rows, :],
                                                in0=t[0:rows, :],
                                                scalar1=0.0, scalar2=1.0,
                                                op0=Alu.max, op1=Alu.min)
                        nc.sync.dma_start(out=out[g][0:rows, c0:c1],
                                          in_=st[0:rows, :])

            for e in range(E):
                sig = pss.tile([SLOTS, PIX], f32, tag="sig", name=f"sig{e}")
                lh = cta[:, 512:512 + SLOTS] if e == 0 else \
                    ctb[:, SLOTS * (e - 1):SLOTS * e]
                nc.tensor.matmul(sig[:, 0:512], lh, bt(0, 512),
                                 start=True, stop=True)
                nc.tensor.matmul(sig[:, 512:1024], lh, bt(512, 1024),
                                 start=True, stop=True)
                # software pipeline with a lag of 2 groups: by the time the
                # img matmuls of group e-2 issue, its exp finished long ago,
                # so the PE never stalls on the ScalarE
                if e >= 2:
                    emit_img(e - 2)
                al = apool.tile([SLOTS, PIX], f16, tag="alpha", name=f"al{e}")
                if e == 0 or e == E - 1:
                    # split the first exp (starts right after the first sig
                    # half-matmul, pulling the whole ScalarE chain earlier)
                    # and the final exp (pipelines the drain chain
                    # sig->exp->img->clamp->dma at 512-col grain)
                    S.activation(al[:, 0:512], sig[:, 0:512], Act.Exp,
                                 scale=-1.0)
                    S.activation(al[:, 512:1024], sig[:, 512:1024], Act.Exp,
                                 scale=-1.0)
                else:
                    S.activation(al, sig, Act.Exp, scale=-1.0)
                alphas[e] = al
            if E >= 2:
                emit_img(E - 2)
            emit_img(E - 1)

    bass_rust.generate_event_semaphores(nc)
    return nc


def _bin_entries(cx, cy, lam):
    """Host-side routing: which gaussians overlap which 32x32 tile."""
    r = np.sqrt(2.0 * SIGMA_CUT * np.maximum(lam, 0.0)) + 1.0

    entries = []  # (frame, ty, tx, index-list)
    for t in range(T):
        x0 = np.clip(((cx[t] - r[t]) // TILE).astype(int), 0, NT - 1)
        x1 = np.clip(((cx[t] + r[t]) // TILE).astype(int), 0, NT - 1)
        y0 = np.clip(((cy[t] - r[t]) // TILE).astype(int), 0, NT - 1)
        y1 = np.clip(((cy[t] + r[t]) // TILE).astype(int), 0, NT - 1)
        buckets = [[[] for _ in range(NT)] for _ in range(NT)]
        for n in range(N):
            for ty in range(y0[n], y1[n] + 1):
                for tx in range(x0[n], x1[n] + 1):
                    buckets[ty][tx].append(n)
        for ty in range(NT):
            for tx in range(NT):
                assert len(buckets[ty][tx]) <= SLOTS, "tile overflow: >128 gaussians"
                entries.append((t, ty, tx, buckets[ty][tx]))
    return entries


def _pack_bins(entries, bins_per_core):
    """Pack tile-entries into groups of <= 128 total slots, <= MAXTILES
    tiles, load-balanced over N_CORES * bins_per_core bins. Returns a list
    of bins, each a list of entry indices, or None if infeasible."""
    nbins = N_CORES * bins_per_core
    order = sorted(range(len(entries)),
                   key=lambda k: -len(entries[k][3]))
    loads = [0] * nbins
    counts = [0] * nbins
    bins = [[] for _ in range(nbins)]
    for k in order:
        occ = len(entries[k][3])
        best = -1
        for b in sorted(range(nbins), key=lambda b: loads[b]):
            if counts[b] < MAXTILES and loads[b] + occ <= SLOTS:
                best = b
                break
        if best < 0:
            return None
        bins[best].append(k)
        loads[best] += occ
        counts[best] += 1
    return bins


def _ensure_ntff_hook():
    """Provide antenv.axon_hooks (missing in this image) so trace=True works."""
    import sys, types, ctypes, contextlib
    if "antenv.axon_hooks" in sys.modules:
        return
    so_path = "/opt/axon/libaxon_pjrt.so"
    if not os.path.exists(so_path):
        return
    lib = ctypes.CDLL(so_path)
    if not hasattr(lib, "axon_start_nrt_profile"):
        return
    lib.axon_start_nrt_profile.argtypes = [ctypes.POINTER(ctypes.c_int64), ctypes.c_size_t]
    lib.axon_start_nrt_profile.restype = ctypes.c_int64
    lib.axon_stop_nrt_profile.argtypes = [ctypes.c_char_p]
    lib.axon_stop_nrt_profile.restype = ctypes.c_int64

    @contextlib.contextmanager
    def _hook(output_dir, device_ids):
        import jax
        jax.devices()
        if device_ids:
            ids = (ctypes.c_int64 * len(device_ids))(*device_ids)
            rc = lib.axon_start_nrt_profile(ids, len(device_ids))
        else:
            rc = lib.axon_start_nrt_profile(None, 0)
        if rc != 0:
            raise RuntimeError(f"axon_start_nrt_profile rc={rc}")
        try:
            yield
        finally:
            n = lib.axon_stop_nrt_profile(str(output_dir).encode())
            print(f"profile: {n} file(s) written to {output_dir}")

    mod = types.ModuleType("antenv.axon_hooks")
    mod.get_axon_ntff_profile_hook = lambda: _hook
    mod.set_axon_ntff_profile_hook = lambda h: None
    sys.modules["antenv.axon_hooks"] = mod


def _split16(c):
    """Split float64 array c into (hi, lo) fp16 with lo pre-scaled by 2^11."""
    hi = c.astype(np.float16)
    lo = ((c - hi.astype(np.float64)) * LO_SCALE).astype(np.float16)
    return hi, lo


def kernel(xyz, cholesky, opacity, features_dc):
    from concourse import bass_utils

    xyz = np.asarray(xyz, np.float32)
    cholesky = np.asarray(cholesky, np.float32)
    opacity = np.asarray(opacity, np.float32)
    features_dc = np.asarray(features_dc, np.float32)

    # ---- host precompute (float64): projection, conic, binning ----
    means = np.tanh(xyz.astype(np.float64))
    cx = 0.5 * W * (means[..., 0] + 1.0)                    # (T,N)
    cy = 0.5 * H * (means[..., 1] + 1.0)
    chol = cholesky.astype(np.float64) + np.array([0.5, 0.0, 0.5])
    l0, l1, l2 = chol[..., 0], chol[..., 1], chol[..., 2]
    sxx, sxy, syy = l0 * l0, l0 * l1, l1 * l1 + l2 * l2
    det = sxx * syy - sxy * sxy
    ca, cb, cc = syy / det, -sxy / det, sxx / det           # conic (T,N)
    tr = sxx + syy
    lam = tr / 2 + np.sqrt(np.maximum(tr * tr / 4 - det, 0.0))

    colors = 1.0 / (1.0 + np.exp(-features_dc.astype(np.float64)))   # (N,3)
    opac = 1.0 / (1.0 + np.exp(-opacity.astype(np.float64)[:, 0]))   # (N,)
    w3 = colors * opac[:, None]                                      # (N,3)

    entries = _bin_entries(cx, cy, lam)
    total = sum(len(e[3]) for e in entries)
    E = max(2, -(-total // (SLOTS * N_CORES)))   # bins per core, lower bound
    bins = None
    while bins is None:
        bins = _pack_bins(entries, E)
        if bins is None:
            E += 1

    # fp16 quadratic basis over local 32x32 pixels; rows 6-11 are the
    # lo-coefficient rows, scaled by 2^-11 (power of two: still exact)
    gx = np.arange(PIX, dtype=np.float64) % TILE
    gy = np.arange(PIX, dtype=np.float64) // TILE
    b6 = np.stack([gx * gx, gx * gy, gy * gy, gx, gy, np.ones(PIX)])
    basis = np.concatenate([b6, b6 / LO_SCALE]).astype(np.float16)

    in_maps = []
    for c in range(N_CORES):
        lm = np.zeros((12, E * SLOTS), np.float16)
        wm = np.zeros((SLOTS, E * 32), np.float16)
        for ei in range(E):
            off = 0
            for j, k in enumerate(bins[c * E + ei]):
                t, ty, tx, idxs = entries[k]
                ns = len(idxs)
                if not ns:
                    continue
                idxs = np.asarray(idxs)
                ex = cx[t, idxs] - tx * TILE
                ey = cy[t, idxs] - ty * TILE
                a_, b_, c_ = ca[t, idxs], cb[t, idxs], cc[t, idxs]
                coef = np.stack([
                    0.5 * a_,
                    b_,
                    0.5 * c_,
                    -(a_ * ex + b_ * ey),
                    -(b_ * ex + c_ * ey),
                    0.5 * (a_ * ex * ex + c_ * ey * ey) + b_ * ex * ey,
                ])                                           # (6, ns)
                hi, lo = _split16(coef)
                s = slice(SLOTS * ei + off, SLOTS * ei + off + ns)
                lm[0:6, s] = hi
                lm[6:12, s] = lo
                wm[off:off + ns, 32 * ei + 3 * j:32 * ei + 3 * j + 3] = \
                    w3[idxs].astype(np.float16)
                off += ns
        cbam = np.concatenate([basis[:, 0:512], lm[:, 0:SLOTS]],
                              axis=1).astype(np.float16)
        cbsm = np.ascontiguousarray(basis[:, 512:1024])
        cbbm = np.ascontiguousarray(lm[:, SLOTS:]) if E > 1 else \
            np.zeros((12, SLOTS), np.float16)
        in_maps.append({"cba": cbam, "cbs": cbsm, "cbb": cbbm, "wtr": wm})

    if E not in _CACHE:
        _CACHE[E] = _build_nc(E)
    nc = _CACHE[E]

    trace = bool(int(os.environ.get("GS_TRACE", "0")))
    if trace:
        _ensure_ntff_hook()
    res = bass_utils.run_bass_kernel_spmd(
        nc, in_maps, core_ids=list(range(N_CORES)), trace=trace)
    kernel.last_result = res

    img = np.zeros((T, 3, H, W), np.float32)
    for c in range(N_CORES):
        o = res.results[c]["out"]     # (G, 128, PIX) fp16
        for ei in range(E):
            g, i = divmod(ei, 4)
            for j, k in enumerate(bins[c * E + ei]):
                t, ty, tx, _ = entries[k]
                blk = o[g, 32 * i + 3 * j:32 * i + 3 * j + 3]
                img[t, :, ty * TILE:(ty + 1) * TILE,
                    tx * TILE:(tx + 1) * TILE] = \
                    blk.reshape(3, TILE, TILE)
    return np.clip(img, 0.0, 1.0)


# revision 55
# speedup vs baseline: 1.0353x; 1.0064x over previous
"""GaussianImage (Cholesky) renderer on 8 trn2 NeuronCores.

Strategy: tile-parallel over the pixel grid with multi-tile slot packing.
The 256x256 image is cut into 32x32-pixel tiles (64/frame, 128 total for
T=2).  The host bins gaussians to tiles (bbox intersect via a
conservative support radius; outside it exp(-sigma) underflows to 0 in
fp32) and then bin-packs several tiles into one 128-slot "group"
(occupancies sum to <= 128; mean tile occupancy is ~40).  All tiles share
the same local 32x32 quadratic basis, so one K=12 fp16 matmul + one Exp
evaluates every gaussian of every tile in the group against all 1024
local pixels:

  sigma = lhsT(12,128)^T @ basis(12,1024)      [TensorE fp16, fp32 PSUM]
  alpha = Exp(-sigma)                          [ScalarE -> fp16]
  img   = w(128,32)^T @ alpha(128,1024)        [TensorE fp16, fp32 PSUM]

The img weights are block-structured: column 3j+c holds channel-c colors
of tile j's gaussians at their slots and zeros elsewhere, so one matmul
renders every tile of the group (row 3j+c = tile j, channel c).  Per-slot
sigma coefficients (quadratic in local pixel coords) are precomputed on
the host and split hi/lo into two fp16 values (lo pre-scaled by 2^11 to
stay in fp16 normal range; matching basis rows scaled by 2^-11), giving
~fp32-accurate sigma at the PE's full fp16 rate (fp32 matmul is 2-4x
slower on the PE; fp16 with exact-integer basis values loses nothing).
Up to 4 groups' images land in per-bank PSUM tiles at partition offsets
32i (PE column-group tiling), are clamped+converted to fp16 by one
VectorE op per 512-col half, and DMA'd out.  Inputs ride four parallel
engine DMA queues; the Exp table is preloaded during the transfers.
Each pixel is owned by exactly one tile -> no cross-core reduction.
"""

import os
import numpy as np

T, N, H, W = 2, 512, 256, 256
TILE = 32
NT = H // TILE          # 8 tiles per axis
N_CORES = 8
SLOTS = 128
PIX = TILE * TILE       # 1024
MAXTILES = 10           # 3*MAXTILES <= 32 img-weight columns per group
SIGMA_CUT = 8.0         # exp(-8) ~ 3e-4 * w: ~1e-3 rel, vs the 2e-2 gate
LO_SCALE = 2048.0       # 2^11: keeps lo-half fp16 coefficients normal

_CACHE = {}


def _build_nc(E):
    import concourse.bass as bass
    import concourse.mybir as mybir
    from concourse.tile import TileContext
    import bass_rust

    f32 = mybir.dt.float32
    f16 = mybir.dt.float16
    Act = mybir.ActivationFunctionType
    Alu = mybir.AluOpType
    G = (E + 3) // 4  # img PSUM tiles, 4 groups each

    nc = bass.Bass("TRN2")
    # Inputs split into four tensors so each rides a different engine's
    # DMA queue and the transfers run in parallel right after engine init:
    #   cba: [basis cols 0:512 | group-0 lhsT coeffs]   (sync queue)
    #   cbs: [basis cols 512:1024]                      (gpsimd queue)
    #   cbb: remaining groups' lhsT coeffs              (sync queue, 2nd)
    #   wtr: img weights                                (scalar queue)
    cba = nc.dram_tensor("cba", [12, 512 + SLOTS], f16, kind="ExternalInput")
    cbs = nc.dram_tensor("cbs", [12, 512], f16, kind="ExternalInput")
    cbb = nc.dram_tensor("cbb", [12, max(E - 1, 1) * SLOTS], f16,
                         kind="ExternalInput")
    wtr = nc.dram_tensor("wtr", [SLOTS, E * 32], f16, kind="ExternalInput")
    out = nc.dram_tensor("out", [G, SLOTS, PIX], f16, kind="ExternalOutput")

    with TileContext(nc) as tc:
        with tc.tile_pool(name="const", bufs=1) as cpool, \
             tc.tile_pool(name="alpha", bufs=4) as apool, \
             tc.tile_pool(name="ps_sig", bufs=3 if G == 1 else 2,
                          space="PSUM") as pss, \
             tc.tile_pool(name="ps_img", bufs=2 if G > 1 else 1,
                          space="PSUM") as psi:

            S = nc.scalar

            # preload the Exp activation table first thing on the scalar
            # queue (the ~1.3us table load otherwise serializes with the
            # first real exp).  The warm tile is read uninitialized on
            # purpose: its output is never consumed.
            warm = cpool.tile([SLOTS, 1], f32, tag="warm")
            S.activation(warm, warm, Act.Exp)

            cta = cpool.tile([12, 512 + SLOTS], f16, tag="cba")
            cts = cpool.tile([12, 512], f16, tag="cbs")
            ctb = cpool.tile([12, max(E - 1, 1) * SLOTS], f16, tag="cbb")
            wt = cpool.tile([SLOTS, E * 32], f16, tag="wtr")
            nc.scalar.dma_start(out=wt, in_=wtr[:])
            nc.sync.dma_start(out=cta, in_=cba[:])
            nc.gpsimd.dma_start(out=cts, in_=cbs[:])
            nc.sync.dma_start(out=ctb, in_=cbb[:])

            def bt(c0, c1):
                return cta[:, c0:c1] if c1 <= 512 else cts[:, c0 - 512:c1 - 512]

            alphas = {}
            imgt = {}

            def emit_img(e):
                g, i = divmod(e, 4)
                al = alphas.pop(e)
                wre = wt[:, 32 * e:32 * e + 32]
                if i == 0:
                    # separate single-bank PSUM tiles per 512-col half so
                    # the drain of one half never serializes against img
                    # matmuls of the other (the bank tracker is per-tile)
                    imgt[g] = (psi.tile([SLOTS, 512], f32, tag="imgA",
                                        name=f"imgA{g}"),
                               psi.tile([SLOTS, 512], f32, tag="imgB",
                                        name=f"imgB{g}"))
                final = e == E - 1
                last = final or i == 3
                for h, (c0, c1) in enumerate(((0, 512), (512, 1024))):
                    t = imgt[g][h]
                    nc.tensor.matmul(t[32 * i:32 * i + 32, :], wre,
                                     al[:, c0:c1], start=True, stop=True,
                                     tile_position=(0, 32 * i))
                    if last:
                        # PSUM->SBUF fp16 copy per half (one op covers all
                        # groups in the block: cost is cols, not rows),
                        # then DMA out only the used rows
                        rows = 32 * (i + 1)
                        st = apool.tile([SLOTS, 512], f16, tag=f"st{h}",
                                        name=f"st{g}_{h}")
                        if final and h == 1:
                            # the very last drain rides the ScalarE (idle
                            # after its exp chain) + scalar DMA queue, so
                            # it never waits for the DVE to finish the
                            # A-half clamp; the host's np.clip covers the
                            # skipped clamping
                            S.activation(st[0:rows, :], t[0:rows, :],
                                         Act.Copy)
                            nc.scalar.dma_start(out=out[g][0:rows, c0:c1],
                                                in_=st[0:rows, :])
                        else:
                            nc.vector.tensor_scalar(out=st[0:rows, :],
                                                    in0=t[0:rows, :],
                                                    scalar1=0.0,
                                                    scalar2=1.0,
                                                    op0=Alu.max,
                                                    op1=Alu.min)
                            nc.sync.dma_start(out=out[g][0:rows, c0:c1],
                                              in_=st[0:rows, :])

            for e in range(E):
                sig = pss.tile([SLOTS, PIX], f32, tag="sig", name=f"sig{e}")
                lh = cta[:, 512:512 + SLOTS] if e == 0 else \
                    ctb[:, SLOTS * (e - 1):SLOTS * e]
                nc.tensor.matmul(sig[:, 0:512], lh, bt(0, 512),
                                 start=True, stop=True)
                nc.tensor.matmul(sig[:, 512:1024], lh, bt(512, 1024),
                                 start=True, stop=True)
                # software pipeline with a lag of 2 groups: by the time the
                # img matmuls of group e-2 issue, its exp finished long ago,
                # so the PE never stalls on the ScalarE
                if e >= 2:
                    emit_img(e - 2)
                al = apool.tile([SLOTS, PIX], f16, tag="alpha", name=f"al{e}")
                if e == 0 or e == E - 1:
                    # split the first exp (starts right after the first sig
                    # half-matmul, pulling the whole ScalarE chain earlier)
                    # and the final exp (pipelines the drain chain
                    # sig->exp->img->clamp->dma at 512-col grain)
                    S.activation(al[:, 0:512], sig[:, 0:512], Act.Exp,
                                 scale=-1.0)
                    S.activation(al[:, 512:1024], sig[:, 512:1024], Act.Exp,
                                 scale=-1.0)
                else:
                    S.activation(al, sig, Act.Exp, scale=-1.0)
                alphas[e] = al
            if E >= 2:
                emit_img(E - 2)
            emit_img(E - 1)

    bass_rust.generate_event_semaphores(nc)
    return nc


def _bin_entries(cx, cy, lam):
    """Host-side routing: which gaussians overlap which 32x32 tile."""
    r = np.sqrt(2.0 * SIGMA_CUT * np.maximum(lam, 0.0)) + 1.0

    entries = []  # (frame, ty, tx, index-list)
    for t in range(T):
        x0 = np.clip(((cx[t] - r[t]) // TILE).astype(int), 0, NT - 1)
        x1 = np.clip(((cx[t] + r[t]) // TILE).astype(int), 0, NT - 1)
        y0 = np.clip(((cy[t] - r[t]) // TILE).astype(int), 0, NT - 1)
        y1 = np.clip(((cy[t] + r[t]) // TILE).astype(int), 0, NT - 1)
        buckets = [[[] for _ in range(NT)] for _ in range(NT)]
        for n in range(N):
            for ty in range(y0[n], y1[n] + 1):
                for tx in range(x0[n], x1[n] + 1):
                    buckets[ty][tx].append(n)
        for ty in range(NT):
            for tx in range(NT):
                assert len(buckets[ty][tx]) <= SLOTS, "tile overflow: >128 gaussians"
                entries.append((t, ty, tx, buckets[ty][tx]))
    return entries


def _pack_bins(entries, bins_per_core):
    """Pack tile-entries into groups of <= 128 total slots, <= MAXTILES
    tiles, load-balanced over N_CORES * bins_per_core bins. Returns a list
    of bins, each a list of entry indices, or None if infeasible."""
    nbins = N_CORES * bins_per_core
    order = sorted(range(len(entries)),
                   key=lambda k: -len(entries[k][3]))
    loads = [0] * nbins
    counts = [0] * nbins
    bins = [[] for _ in range(nbins)]
    for k in order:
        occ = len(entries[k][3])
        best = -1
        for b in sorted(range(nbins), key=lambda b: loads[b]):
            if counts[b] < MAXTILES and loads[b] + occ <= SLOTS:
                best = b
                break
        if best < 0:
            return None
        bins[best].append(k)
        loads[best] += occ
        counts[best] += 1
    return bins


def _ensure_ntff_hook():
    """Provide antenv.axon_hooks (missing in this image) so trace=True works."""
    import sys, types, ctypes, contextlib
    if "antenv.axon_hooks" in sys.modules:
        return
    so_path = "/opt/axon/libaxon_pjrt.so"
    if not os.path.exists(so_path):
        return
    lib = ctypes.CDLL(so_path)
    if not hasattr(lib, "axon_start_nrt_profile"):
        return
    lib.axon_start_nrt_profile.argtypes = [ctypes.POINTER(ctypes.c_int64), ctypes.c_size_t]
    lib.axon_start_nrt_profile.restype = ctypes.c_int64
    lib.axon_stop_nrt_profile.argtypes = [ctypes.c_char_p]
    lib.axon_stop_nrt_profile.restype = ctypes.c_int64

    @contextlib.contextmanager
    def _hook(output_dir, device_ids):
        import jax
        jax.devices()
        if device_ids:
            ids = (ctypes.c_int64 * len(device_ids))(*device_ids)
            rc = lib.axon_start_nrt_profile(ids, len(device_ids))
        else:
            rc = lib.axon_start_nrt_profile(None, 0)
        if rc != 0:
            raise RuntimeError(f"axon_start_nrt_profile rc={rc}")
        try:
            yield
        finally:
            n = lib.axon_stop_nrt_profile(str(output_dir).encode())
            print(f"profile: {n} file(s) written to {output_dir}")

    mod = types.ModuleType("antenv.axon_hooks")
    mod.get_axon_ntff_profile_hook = lambda: _hook
    mod.set_axon_ntff_profile_hook = lambda h: None
    sys.modules["antenv.axon_hooks"] = mod


def _split16(c):
    """Split float64 array c into (hi, lo) fp16 with lo pre-scaled by 2^11."""
    hi = c.astype(np.float16)
    lo = ((c - hi.astype(np.float64)) * LO_SCALE).astype(np.float16)
    return hi, lo


def kernel(xyz, cholesky, opacity, features_dc):
    from concourse import bass_utils

    xyz = np.asarray(xyz, np.float32)
    cholesky = np.asarray(cholesky, np.float32)
    opacity = np.asarray(opacity, np.float32)
    features_dc = np.asarray(features_dc, np.float32)

    # ---- host precompute (float64): projection, conic, binning ----
    means = np.tanh(xyz.astype(np.float64))
    cx = 0.5 * W * (means[..., 0] + 1.0)                    # (T,N)
    cy = 0.5 * H * (means[..., 1] + 1.0)
    chol = cholesky.astype(np.float64) + np.array([0.5, 0.0, 0.5])
    l0, l1, l2 = chol[..., 0], chol[..., 1], chol[..., 2]
    sxx, sxy, syy = l0 * l0, l0 * l1, l1 * l1 + l2 * l2
    det = sxx * syy - sxy * sxy
    ca, cb, cc = syy / det, -sxy / det, sxx / det           # conic (T,N)
    tr = sxx + syy
    lam = tr / 2 + np.sqrt(np.maximum(tr * tr / 4 - det, 0.0))

    colors = 1.0 / (1.0 + np.exp(-features_dc.astype(np.float64)))   # (N,3)
    opac = 1.0 / (1.0 + np.exp(-opacity.astype(np.float64)[:, 0]))   # (N,)
    w3 = colors * opac[:, None]                                      # (N,3)

    entries = _bin_entries(cx, cy, lam)
    total = sum(len(e[3]) for e in entries)
    E = max(2, -(-total // (SLOTS * N_CORES)))   # bins per core, lower bound
    bins = None
    while bins is None:
        bins = _pack_bins(entries, E)
        if bins is None:
            E += 1

    # fp16 quadratic basis over local 32x32 pixels; rows 6-11 are the
    # lo-coefficient rows, scaled by 2^-11 (power of two: still exact)
    gx = np.arange(PIX, dtype=np.float64) % TILE
    gy = np.arange(PIX, dtype=np.float64) // TILE
    b6 = np.stack([gx * gx, gx * gy, gy * gy, gx, gy, np.ones(PIX)])
    basis = np.concatenate([b6, b6 / LO_SCALE]).astype(np.float16)

    in_maps = []
    for c in range(N_CORES):
        lm = np.zeros((12, E * SLOTS), np.float16)
        wm = np.zeros((SLOTS, E * 32), np.float16)
        for ei in range(E):
            off = 0
            for j, k in enumerate(bins[c * E + ei]):
                t, ty, tx, idxs = entries[k]
                ns = len(idxs)
                if not ns:
                    continue
                idxs = np.asarray(idxs)
                ex = cx[t, idxs] - tx * TILE
                ey = cy[t, idxs] - ty * TILE
                a_, b_, c_ = ca[t, idxs], cb[t, idxs], cc[t, idxs]
                coef = np.stack([
                    0.5 * a_,
                    b_,
                    0.5 * c_,
                    -(a_ * ex + b_ * ey),
                    -(b_ * ex + c_ * ey),
                    0.5 * (a_ * ex * ex + c_ * ey * ey) + b_ * ex * ey,
                ])                                           # (6, ns)
                hi, lo = _split16(coef)
                s = slice(SLOTS * ei + off, SLOTS * ei + off + ns)
                lm[0:6, s] = hi
                lm[6:12, s] = lo
                wm[off:off + ns, 32 * ei + 3 * j:32 * ei + 3 * j + 3] = \
                    w3[idxs].astype(np.float16)
                off += ns
        cbam = np.concatenate([basis[:, 0:512], lm[:, 0:SLOTS]],
                              axis=1).astype(np.float16)
        cbsm = np.ascontiguousarray(basis[:, 512:1024])
        cbbm = np.ascontiguousarray(lm[:, SLOTS:]) if E > 1 else \
            np.zeros((12, SLOTS), np.float16)
        in_maps.append({"cba": cbam, "cbs": cbsm, "cbb": cbbm, "wtr": wm})

    if E not in _CACHE:
        _CACHE[E] = _build_nc(E)
    nc = _CACHE[E]

    trace = bool(int(os.environ.get("GS_TRACE", "0")))
    if trace:
        _ensure_ntff_hook()
    res = bass_utils.run_bass_kernel_spmd(
        nc, in_maps, core_ids=list(range(N_CORES)), trace=trace)
    kernel.last_result = res

    img = np.zeros((T, 3, H, W), np.float32)
    for c in range(N_CORES):
        o = res.results[c]["out"]     # (G, 128, PIX) fp16
        for ei in range(E):
            g, i = divmod(ei, 4)
            for j, k in enumerate(bins[c * E + ei]):
                t, ty, tx, _ = entries[k]
                blk = o[g, 32 * i + 3 * j:32 * i + 3 * j + 3]
                img[t, :, ty * TILE:(ty + 1) * TILE,
                    tx * TILE:(tx + 1) * TILE] = \
                    blk.reshape(3, TILE, TILE)
    return np.clip(img, 0.0, 1.0)


# revision 56
# speedup vs baseline: 1.0517x; 1.0158x over previous
"""GaussianImage (Cholesky) renderer on 8 trn2 NeuronCores.

Strategy: tile-parallel over the pixel grid with multi-tile slot packing.
The 256x256 image is cut into 32x32-pixel tiles (64/frame, 128 total for
T=2).  The host bins gaussians to tiles (bbox intersect via a
conservative support radius; outside it exp(-sigma) underflows to 0 in
fp32) and then bin-packs several tiles into one 128-slot "group"
(occupancies sum to <= 128; mean tile occupancy is ~40).  All tiles share
the same local 32x32 quadratic basis, so one K=12 fp16 matmul + one Exp
evaluates every gaussian of every tile in the group against all 1024
local pixels:

  sigma = lhsT(12,128)^T @ basis(12,1024)      [TensorE fp16, fp32 PSUM]
  alpha = Exp(-sigma)                          [ScalarE -> fp16]
  img   = w(128,32)^T @ alpha(128,1024)        [TensorE fp16, fp32 PSUM]

The img weights are block-structured: column 3j+c holds channel-c colors
of tile j's gaussians at their slots and zeros elsewhere, so one matmul
renders every tile of the group (row 3j+c = tile j, channel c).  Per-slot
sigma coefficients (quadratic in local pixel coords) are precomputed on
the host and split hi/lo into two fp16 values (lo pre-scaled by 2^11 to
stay in fp16 normal range; matching basis rows scaled by 2^-11), giving
~fp32-accurate sigma at the PE's full fp16 rate (fp32 matmul is 2-4x
slower on the PE; fp16 with exact-integer basis values loses nothing).
Up to 4 groups' images land in per-bank PSUM tiles at partition offsets
32i (PE column-group tiling) and drain via two parallel chains: the A
half clamps on the VectorE (one op per block: cost is cols, not rows)
with a sync-queue DMA, while the final B half copies on the ScalarE
(idle after its exp chain; the host's np.clip supplies the clamp) with
a scalar-queue DMA.  Inputs ride parallel engine DMA queues; the Exp
table is preloaded during the transfers.  Each pixel is owned by
exactly one tile -> no cross-core reduction.
"""

import os
import numpy as np

T, N, H, W = 2, 512, 256, 256
TILE = 32
NT = H // TILE          # 8 tiles per axis
N_CORES = 8
SLOTS = 128
PIX = TILE * TILE       # 1024
MAXTILES = 10           # 3*MAXTILES <= 32 img-weight columns per group
SIGMA_CUT = 8.0         # exp(-8) ~ 3e-4 * w: ~1e-3 rel, vs the 2e-2 gate
LO_SCALE = 2048.0       # 2^11: keeps lo-half fp16 coefficients normal

_CACHE = {}


def _build_nc(E):
    import concourse.bass as bass
    import concourse.mybir as mybir
    from concourse.tile import TileContext
    import bass_rust

    f32 = mybir.dt.float32
    f16 = mybir.dt.float16
    Act = mybir.ActivationFunctionType
    Alu = mybir.AluOpType
    G = (E + 3) // 4  # img PSUM tiles, 4 groups each

    nc = bass.Bass("TRN2")
    # Inputs split into four tensors so each rides a different engine's
    # DMA queue and the transfers run in parallel right after engine init:
    #   cba: [basis cols 0:512 | group-0 lhsT coeffs]   (sync queue)
    #   cbs: [basis cols 512:1024]                      (gpsimd queue)
    #   cbb: remaining groups' lhsT coeffs              (sync queue, 2nd)
    #   wtr: img weights                                (scalar queue)
    cba = nc.dram_tensor("cba", [12, 512 + SLOTS], f16, kind="ExternalInput")
    cbs = nc.dram_tensor("cbs", [12, 512], f16, kind="ExternalInput")
    cbb = nc.dram_tensor("cbb", [12, max(E - 1, 1) * SLOTS], f16,
                         kind="ExternalInput")
    wtr = nc.dram_tensor("wtr", [SLOTS, E * 32], f16, kind="ExternalInput")
    out = nc.dram_tensor("out", [G, SLOTS, PIX], f16, kind="ExternalOutput")

    with TileContext(nc) as tc:
        with tc.tile_pool(name="const", bufs=1) as cpool, \
             tc.tile_pool(name="alpha", bufs=4) as apool, \
             tc.tile_pool(name="ps_sig", bufs=3 if G == 1 else 2,
                          space="PSUM") as pss, \
             tc.tile_pool(name="ps_img", bufs=2 if G > 1 else 1,
                          space="PSUM") as psi:

            S = nc.scalar

            # preload the Exp activation table first thing on the scalar
            # queue (the ~1.3us table load otherwise serializes with the
            # first real exp).  The warm tile is read uninitialized on
            # purpose: its output is never consumed.
            warm = cpool.tile([SLOTS, 1], f32, tag="warm")
            S.activation(warm, warm, Act.Exp)

            cta = cpool.tile([12, 512 + SLOTS], f16, tag="cba")
            cts = cpool.tile([12, 512], f16, tag="cbs")
            ctb = cpool.tile([12, max(E - 1, 1) * SLOTS], f16, tag="cbb")
            wt = cpool.tile([SLOTS, E * 32], f16, tag="wtr")
            nc.scalar.dma_start(out=wt, in_=wtr[:])
            nc.sync.dma_start(out=cta, in_=cba[:])
            nc.gpsimd.dma_start(out=cts, in_=cbs[:])
            nc.sync.dma_start(out=ctb, in_=cbb[:])

            def bt(c0, c1):
                return cta[:, c0:c1] if c1 <= 512 else cts[:, c0 - 512:c1 - 512]

            alphas = {}
            imgt = {}

            def emit_img(e):
                g, i = divmod(e, 4)
                al = alphas.pop(e)
                wre = wt[:, 32 * e:32 * e + 32]
                if i == 0:
                    # separate single-bank PSUM tiles per 512-col half so
                    # the drain of one half never serializes against img
                    # matmuls of the other (the bank tracker is per-tile)
                    imgt[g] = (psi.tile([SLOTS, 512], f32, tag="imgA",
                                        name=f"imgA{g}"),
                               psi.tile([SLOTS, 512], f32, tag="imgB",
                                        name=f"imgB{g}"))
                final = e == E - 1
                last = final or i == 3
                for h, (c0, c1) in enumerate(((0, 512), (512, 1024))):
                    t = imgt[g][h]
                    nc.tensor.matmul(t[32 * i:32 * i + 32, :], wre,
                                     al[:, c0:c1], start=True, stop=True,
                                     tile_position=(0, 32 * i))
                    if last:
                        # PSUM->SBUF fp16 copy per half (one op covers all
                        # groups in the block: cost is cols, not rows),
                        # then DMA out only the used rows
                        rows = 32 * (i + 1)
                        st = apool.tile([SLOTS, 512], f16, tag=f"st{h}",
                                        name=f"st{g}_{h}")
                        if final and h == 1:
                            # the very last drain rides the ScalarE (idle
                            # after its exp chain) + scalar DMA queue, so
                            # it never waits for the DVE to finish the
                            # A-half clamp; the host's np.clip covers the
                            # skipped clamping
                            S.activation(st[0:rows, :], t[0:rows, :],
                                         Act.Copy)
                            nc.scalar.dma_start(out=out[g][0:rows, c0:c1],
                                                in_=st[0:rows, :])
                        else:
                            nc.vector.tensor_scalar(out=st[0:rows, :],
                                                    in0=t[0:rows, :],
                                                    scalar1=0.0,
                                                    scalar2=1.0,
                                                    op0=Alu.max,
                                                    op1=Alu.min)
                            nc.sync.dma_start(out=out[g][0:rows, c0:c1],
                                              in_=st[0:rows, :])

            for e in range(E):
                sig = pss.tile([SLOTS, PIX], f32, tag="sig", name=f"sig{e}")
                lh = cta[:, 512:512 + SLOTS] if e == 0 else \
                    ctb[:, SLOTS * (e - 1):SLOTS * e]
                nc.tensor.matmul(sig[:, 0:512], lh, bt(0, 512),
                                 start=True, stop=True)
                nc.tensor.matmul(sig[:, 512:1024], lh, bt(512, 1024),
                                 start=True, stop=True)
                # software pipeline with a lag of 2 groups: by the time the
                # img matmuls of group e-2 issue, its exp finished long ago,
                # so the PE never stalls on the ScalarE
                if e >= 2:
                    emit_img(e - 2)
                al = apool.tile([SLOTS, PIX], f16, tag="alpha", name=f"al{e}")
                if e == 0 or e == E - 1:
                    # split the first exp (starts right after the first sig
                    # half-matmul, pulling the whole ScalarE chain earlier)
                    # and the final exp (pipelines the drain chain
                    # sig->exp->img->clamp->dma at 512-col grain)
                    S.activation(al[:, 0:512], sig[:, 0:512], Act.Exp,
                                 scale=-1.0)
                    S.activation(al[:, 512:1024], sig[:, 512:1024], Act.Exp,
                                 scale=-1.0)
                else:
                    S.activation(al, sig, Act.Exp, scale=-1.0)
                alphas[e] = al
            if E >= 2:
                emit_img(E - 2)
            emit_img(E - 1)

    bass_rust.generate_event_semaphores(nc)
    return nc


def _bin_entries(cx, cy, lam):
    """Host-side routing: which gaussians overlap which 32x32 tile."""
    r = np.sqrt(2.0 * SIGMA_CUT * np.maximum(lam, 0.0)) + 1.0

    entries = []  # (frame, ty, tx, index-list)
    for t in range(T):
        x0 = np.clip(((cx[t] - r[t]) // TILE).astype(int), 0, NT - 1)
        x1 = np.clip(((cx[t] + r[t]) // TILE).astype(int), 0, NT - 1)
        y0 = np.clip(((cy[t] - r[t]) // TILE).astype(int), 0, NT - 1)
        y1 = np.clip(((cy[t] + r[t]) // TILE).astype(int), 0, NT - 1)
        buckets = [[[] for _ in range(NT)] for _ in range(NT)]
        for n in range(N):
            for ty in range(y0[n], y1[n] + 1):
                for tx in range(x0[n], x1[n] + 1):
                    buckets[ty][tx].append(n)
        for ty in range(NT):
            for tx in range(NT):
                assert len(buckets[ty][tx]) <= SLOTS, "tile overflow: >128 gaussians"
                entries.append((t, ty, tx, buckets[ty][tx]))
    return entries


def _pack_bins(entries, bins_per_core):
    """Pack tile-entries into groups of <= 128 total slots, <= MAXTILES
    tiles, load-balanced over N_CORES * bins_per_core bins. Returns a list
    of bins, each a list of entry indices, or None if infeasible."""
    nbins = N_CORES * bins_per_core
    order = sorted(range(len(entries)),
                   key=lambda k: -len(entries[k][3]))
    loads = [0] * nbins
    counts = [0] * nbins
    bins = [[] for _ in range(nbins)]
    for k in order:
        occ = len(entries[k][3])
        best = -1
        for b in sorted(range(nbins), key=lambda b: loads[b]):
            if counts[b] < MAXTILES and loads[b] + occ <= SLOTS:
                best = b
                break
        if best < 0:
            return None
        bins[best].append(k)
        loads[best] += occ
        counts[best] += 1
    return bins


def _ensure_ntff_hook():
    """Provide antenv.axon_hooks (missing in this image) so trace=True works."""
    import sys, types, ctypes, contextlib
    if "antenv.axon_hooks" in sys.modules:
        return
    so_path = "/opt/axon/libaxon_pjrt.so"
    if not os.path.exists(so_path):
        return
    lib = ctypes.CDLL(so_path)
    if not hasattr(lib, "axon_start_nrt_profile"):
        return
    lib.axon_start_nrt_profile.argtypes = [ctypes.POINTER(ctypes.c_int64), ctypes.c_size_t]
    lib.axon_start_nrt_profile.restype = ctypes.c_int64
    lib.axon_stop_nrt_profile.argtypes = [ctypes.c_char_p]
    lib.axon_stop_nrt_profile.restype = ctypes.c_int64

    @contextlib.contextmanager
    def _hook(output_dir, device_ids):
        import jax
        jax.devices()
        if device_ids:
            ids = (ctypes.c_int64 * len(device_ids))(*device_ids)
            rc = lib.axon_start_nrt_profile(ids, len(device_ids))
        else:
            rc = lib.axon_start_nrt_profile(None, 0)
        if rc != 0:
            raise RuntimeError(f"axon_start_nrt_profile rc={rc}")
        try:
            yield
        finally:
            n = lib.axon_stop_nrt_profile(str(output_dir).encode())
            print(f"profile: {n} file(s) written to {output_dir}")

    mod = types.ModuleType("antenv.axon_hooks")
    mod.get_axon_ntff_profile_hook = lambda: _hook
    mod.set_axon_ntff_profile_hook = lambda h: None
    sys.modules["antenv.axon_hooks"] = mod


def _split16(c):
    """Split float64 array c into (hi, lo) fp16 with lo pre-scaled by 2^11."""
    hi = c.astype(np.float16)
    lo = ((c - hi.astype(np.float64)) * LO_SCALE).astype(np.float16)
    return hi, lo


def kernel(xyz, cholesky, opacity, features_dc):
    from concourse import bass_utils

    xyz = np.asarray(xyz, np.float32)
    cholesky = np.asarray(cholesky, np.float32)
    opacity = np.asarray(opacity, np.float32)
    features_dc = np.asarray(features_dc, np.float32)

    # ---- host precompute (float64): projection, conic, binning ----
    means = np.tanh(xyz.astype(np.float64))
    cx = 0.5 * W * (means[..., 0] + 1.0)                    # (T,N)
    cy = 0.5 * H * (means[..., 1] + 1.0)
    chol = cholesky.astype(np.float64) + np.array([0.5, 0.0, 0.5])
    l0, l1, l2 = chol[..., 0], chol[..., 1], chol[..., 2]
    sxx, sxy, syy = l0 * l0, l0 * l1, l1 * l1 + l2 * l2
    det = sxx * syy - sxy * sxy
    ca, cb, cc = syy / det, -sxy / det, sxx / det           # conic (T,N)
    tr = sxx + syy
    lam = tr / 2 + np.sqrt(np.maximum(tr * tr / 4 - det, 0.0))

    colors = 1.0 / (1.0 + np.exp(-features_dc.astype(np.float64)))   # (N,3)
    opac = 1.0 / (1.0 + np.exp(-opacity.astype(np.float64)[:, 0]))   # (N,)
    w3 = colors * opac[:, None]                                      # (N,3)

    entries = _bin_entries(cx, cy, lam)
    total = sum(len(e[3]) for e in entries)
    E = max(2, -(-total // (SLOTS * N_CORES)))   # bins per core, lower bound
    bins = None
    while bins is None:
        bins = _pack_bins(entries, E)
        if bins is None:
            E += 1

    # fp16 quadratic basis over local 32x32 pixels; rows 6-11 are the
    # lo-coefficient rows, scaled by 2^-11 (power of two: still exact)
    gx = np.arange(PIX, dtype=np.float64) % TILE
    gy = np.arange(PIX, dtype=np.float64) // TILE
    b6 = np.stack([gx * gx, gx * gy, gy * gy, gx, gy, np.ones(PIX)])
    basis = np.concatenate([b6, b6 / LO_SCALE]).astype(np.float16)

    in_maps = []
    for c in range(N_CORES):
        lm = np.zeros((12, E * SLOTS), np.float16)
        wm = np.zeros((SLOTS, E * 32), np.float16)
        for ei in range(E):
            off = 0
            for j, k in enumerate(bins[c * E + ei]):
                t, ty, tx, idxs = entries[k]
                ns = len(idxs)
                if not ns:
                    continue
                idxs = np.asarray(idxs)
                ex = cx[t, idxs] - tx * TILE
                ey = cy[t, idxs] - ty * TILE
                a_, b_, c_ = ca[t, idxs], cb[t, idxs], cc[t, idxs]
                coef = np.stack([
                    0.5 * a_,
                    b_,
                    0.5 * c_,
                    -(a_ * ex + b_ * ey),
                    -(b_ * ex + c_ * ey),
                    0.5 * (a_ * ex * ex + c_ * ey * ey) + b_ * ex * ey,
                ])                                           # (6, ns)
                hi, lo = _split16(coef)
                s = slice(SLOTS * ei + off, SLOTS * ei + off + ns)
                lm[0:6, s] = hi
                lm[6:12, s] = lo
                wm[off:off + ns, 32 * ei + 3 * j:32 * ei + 3 * j + 3] = \
                    w3[idxs].astype(np.float16)
                off += ns
        cbam = np.concatenate([basis[:, 0:512], lm[:, 0:SLOTS]],
                              axis=1).astype(np.float16)
        cbsm = np.ascontiguousarray(basis[:, 512:1024])
        cbbm = np.ascontiguousarray(lm[:, SLOTS:]) if E > 1 else \
            np.zeros((12, SLOTS), np.float16)
        in_maps.append({"cba": cbam, "cbs": cbsm, "cbb": cbbm, "wtr": wm})

    if E not in _CACHE:
        _CACHE[E] = _build_nc(E)
    nc = _CACHE[E]

    trace = bool(int(os.environ.get("GS_TRACE", "0")))
    if trace:
        _ensure_ntff_hook()
    res = bass_utils.run_bass_kernel_spmd(
        nc, in_maps, core_ids=list(range(N_CORES)), trace=trace)
    kernel.last_result = res

    img = np.zeros((T, 3, H, W), np.float32)
    for c in range(N_CORES):
        o = res.results[c]["out"]     # (G, 128, PIX) fp16
        for ei in range(E):
            g, i = divmod(ei, 4)
            for j, k in enumerate(bins[c * E + ei]):
                t, ty, tx, _ = entries[k]
                blk = o[g, 32 * i + 3 * j:32 * i + 3 * j + 3]
                img[t, :, ty * TILE:(ty + 1) * TILE,
                    tx * TILE:(tx + 1) * TILE] = \
                    blk.reshape(3, TILE, TILE)
    return np.clip(img, 0.0, 1.0)
